# revision 7
# baseline (speedup 1.0000x reference)
"""MetaSR super-resolution kernel for 8 Trainium2 NeuronCores (Bass/Tile).

Shard: core = (batch b, query-half h); each core handles one 64x64x64 feature
map and 8192 queries.

Per-core pipeline:
  A. query prep (DVE, exact fp32 replica of the reference index math)
       -> gather indices `lin`, MLP inputs inpT = [rel_y, rel_x, r_rev, 1]
  I. gather-index layout build (wrapped int16, replicated over 8 Q7 cores)
  C. mm1 (PE, fp16): hdd[q, 256] = relu(inp @ w1 + b1)  (b1 via ones row)
  B. feat9: 5 SBUF tiles [128, 4096] fp16 holding the 3x3 unfold of feat
       (free-dim shifted copies + zeroed borders), k' = t*64 + c
  D. GP (PE, fp16): GP[p, o*256+h] = sum_k feat_u[p, k'] * W2'[k', (o,h)]
       for all 4096 spatial positions; +3 cols of B[p,o] = feat_u @ b2' when
       b2 != 0.  Streamed to a DRAM table.
  E. dma_gather (GPSIMD): per-query rows GP[lin_q] -> [128, slot, JW] fp16
     tensor_tensor_reduce (DVE): pred[q,o] = sum_h hdd[q,h]*GP[lin_q, o*256+h]
       (+ B[lin_q,o] as the reduce's per-partition initial value)
"""
import sys
sys.path.insert(0, "/opt/trn_rl_repo")
from contextlib import ExitStack

import numpy as np
import concourse.bass as bass
import concourse.bacc as bacc
import concourse.mybir as mybir
import concourse.tile as tile
from concourse.bass_utils import run_bass_kernel_spmd

AL = mybir.AluOpType
AF = mybir.ActivationFunctionType
F32, F16, I16 = mybir.dt.float32, mybir.dt.float16, mybir.dt.int16

C, H, W = 64, 64, 64
HW = H * W                  # 4096
QC = 8192                   # queries per core
HID = 256
EPS = 1e-6
NB = 4                      # gather batches
GB = QC // NB               # 2048 per gather


def build_nc(has_b2: bool, num_devices: int = 8, stage: str = "AICBDE"):
    JW = 896 if has_b2 else 768     # GP row width (o-major 3*256, + b2 cols)
    nc = bacc.Bacc("TRN2", target_bir_lowering=False, debug=False,
                   num_devices=num_devices)
    featb = nc.dram_tensor("featb", [C, HW], F32, kind="ExternalInput")
    coords = nc.dram_tensor("coords", [128, 128], F32, kind="ExternalInput")
    cells = nc.dram_tensor("cells", [128, 128], F32, kind="ExternalInput")
    w1a = nc.dram_tensor("w1a", [4, HID], F32, kind="ExternalInput")
    w2p = nc.dram_tensor("w2p", [640, JW], F16, kind="ExternalInput")
    pred_d = nc.dram_tensor("pred", [QC, 3], F32, kind="ExternalOutput")
    # scratch DRAM
    gp_d = nc.dram_tensor("gp_scr", [HW, JW], F16, kind="ExternalOutput")
    inpT_d = nc.dram_tensor("inpT_scr", [3, QC], F32, kind="Internal")
    lin_d = nc.dram_tensor("lin_scr", [1, QC], I16, kind="Internal")

    es = ExitStack()
    gsems = [es.enter_context(nc.semaphore(f"gsem{i}")) for i in range(NB)]

    with tile.TileContext(nc) as tc:
        with tc.tile_pool(name="main", bufs=1) as mp:
            pred_sb = mp.tile([128, 64, 3], F32)
            if "E" not in stage:
                nc.vector.memset(pred_sb[:], 0.0)

            # ---------------- Phase A: query prep ----------------
            if "A" in stage:
                with tc.tile_pool(name="prep", bufs=1) as pp:
                    cr = pp.tile([128, 128], F32)
                    nc.sync.dma_start(cr[:], coords.ap())
                    ce = pp.tile([128, 128], F32)
                    nc.sync.dma_start(ce[:], cells.ap())
                    # coord_ = coord - cell/2   (exact: cell*0.5 then sub)
                    half = pp.tile([128, 128], F32)
                    nc.vector.tensor_scalar(half[:], ce[:], 0.5, None, AL.mult)
                    co = pp.tile([128, 128], F32)
                    nc.vector.tensor_tensor(co[:], cr[:], half[:], AL.subtract)
                    # cq = clip(coord_ + EPS, -1+EPS, 1-EPS)
                    cq = pp.tile([128, 128], F32)
                    nc.vector.tensor_scalar(cq[:], co[:], EPS, -1.0 + EPS, AL.add, AL.max)
                    nc.vector.tensor_scalar(cq[:], cq[:], 1.0 - EPS, None, AL.min)
                    # t = ((cq + 1)*64 - 1) * 0.5
                    t = pp.tile([128, 128], F32)
                    nc.vector.tensor_scalar(t[:], cq[:], 1.0, None, AL.add)
                    nc.vector.tensor_scalar(t[:], t[:], 64.0, -1.0, AL.mult, AL.add)
                    nc.vector.tensor_scalar(t[:], t[:], 0.5, None, AL.mult)
                    # round-half-even via +-2^23, then clip to [0, 63]
                    M = 8388608.0
                    nc.vector.tensor_scalar(t[:], t[:], M, None, AL.add)
                    nc.vector.tensor_scalar(t[:], t[:], M, None, AL.subtract)
                    nc.vector.tensor_scalar(t[:], t[:], 0.0, 63.0, AL.max, AL.min)
                    # q_coord = iyx/32 - 1 ; rel = (coord_ - q_coord) * 32
                    qc_ = pp.tile([128, 128], F32)
                    nc.vector.tensor_scalar(qc_[:], t[:], 0.03125, -1.0, AL.mult, AL.add)
                    rel = pp.tile([128, 128], F32)
                    nc.vector.tensor_tensor(rel[:], co[:], qc_[:], AL.subtract)
                    nc.vector.tensor_scalar(rel[:], rel[:], 32.0, None, AL.mult)
                    # r_rev = cell_y * 32
                    rrev = pp.tile([128, 64], F32)
                    nc.vector.tensor_scalar(rrev[:], ce[:, 0:128:2], 32.0, None, AL.mult)
                    # lin = iy*64 + ix  (fp32 exact), -> int16
                    linf = pp.tile([128, 64], F32)
                    nc.vector.scalar_tensor_tensor(
                        linf[:], t[:, 0:128:2], 64.0, t[:, 1:128:2], AL.mult, AL.add)
                    lin16 = pp.tile([128, 64], I16)
                    nc.vector.tensor_copy(lin16[:], linf[:])
                    # DMA out: inpT rows + lin
                    nc.sync.dma_start(
                        inpT_d.ap()[0:1, :].rearrange("o (p f) -> o p f", p=128),
                        rel[:, 0:128:2])
                    nc.sync.dma_start(
                        inpT_d.ap()[1:2, :].rearrange("o (p f) -> o p f", p=128),
                        rel[:, 1:128:2])
                    nc.sync.dma_start(
                        inpT_d.ap()[2:3, :].rearrange("o (p f) -> o p f", p=128),
                        rrev[:])
                    nc.sync.dma_start(
                        lin_d.ap().rearrange("o (p f) -> o p f", p=128), lin16[:])

            # ---------------- Phase I: gather index layout ----------------
            idx_sb = mp.tile([128, QC // 16], I16)
            if "I" in stage:
                for g in range(8):
                    nc.sync.dma_start(
                        idx_sb[16 * g:16 * (g + 1), :],
                        lin_d.ap().rearrange("o (f p) -> o p f", p=16)[0])

            # ---------------- Phase C: mm1 (hdd) ----------------
            hdd = mp.tile([128, 64, HID], F16)
            if "C" in stage:
                # inpT with ones row (fp16; gpsimd DMA casts fp32 -> fp16)
                inpT = mp.tile([4, QC], F16)
                nc.vector.memset(inpT[:], 1.0)
                nc.gpsimd.dma_start(inpT[0:3, :], inpT_d.ap())
                w1s = mp.tile([4, HID], F16)
                nc.gpsimd.dma_start(w1s[:], w1a.ap())
                with tc.tile_pool(name="ps1", bufs=2, space="PSUM") as ps1:
                    for k in range(64):
                        hp = ps1.tile([128, HID], F32, tag="hp")
                        nc.tensor.matmul(hp[:],
                                         inpT[:, 128 * k:128 * (k + 1)],
                                         w1s[:], start=True, stop=True)
                        nc.scalar.activation(hdd[:, k, :], hp[:], AF.Relu)

            # ---------------- Phase B: feat9 ----------------
            f9 = []
            if "B" in stage:
                for kc in range(5):
                    f9t = mp.tile([128, HW], F16, name=f"f9_{kc}")
                    f9.append(f9t)
                with tc.tile_pool(name="fb", bufs=1) as fb:
                    f2 = fb.tile([128, HW], F32)
                    nc.sync.dma_start(f2[0:64, :], featb.ap())
                    nc.sync.dma_start(f2[64:128, :], featb.ap())
                    f16 = fb.tile([128, HW], F16)
                    nc.vector.tensor_copy(f16[:], f2[:])
                    for kc in range(5):
                        nc.vector.memset(f9[kc][:], 0.0)
                        for hh in range(2):
                            tt = 2 * kc + hh
                            if tt > 8:
                                continue
                            dy, dx = tt // 3 - 1, tt % 3 - 1
                            off = dy * 64 + dx
                            lo, hi = max(0, -off), HW - max(0, off)
                            sl = slice(64 * hh, 64 * (hh + 1))
                            nc.vector.tensor_copy(f9[kc][sl, lo:hi],
                                                  f16[sl, lo + off:hi + off])
                            if dx == -1:
                                nc.vector.memset(
                                    f9[kc][sl].rearrange("p (y x) -> p y x", x=64)[:, :, 0:1], 0.0)
                            elif dx == 1:
                                nc.vector.memset(
                                    f9[kc][sl].rearrange("p (y x) -> p y x", x=64)[:, :, 63:64], 0.0)

            # ---------------- Phase D: GP table ----------------
            if "D" in stage:
                w2s = mp.tile([128, 5, JW], F16)
                nc.sync.dma_start(
                    w2s[:], w2p.ap().rearrange("(kc p) j -> p kc j", p=128))
                with tc.tile_pool(name="gpb", bufs=2) as gpb, \
                     tc.tile_pool(name="ps2", bufs=2, space="PSUM") as ps2:
                    jchunks = [(0, 512), (512, JW)]
                    for pt in range(32):
                        gps = ps2.tile([128, JW], F32, tag="gps")
                        for (j0, j1) in jchunks:
                            for kc in range(5):
                                nc.tensor.matmul(gps[:, j0:j1],
                                                 f9[kc][:, 128 * pt:128 * (pt + 1)],
                                                 w2s[:, kc, j0:j1],
                                                 start=(kc == 0), stop=(kc == 4))
                        gsb = gpb.tile([128, JW], F16, tag="gsb")
                        nc.scalar.activation(gsb[:], gps[:], AF.Copy)
                        nc.sync.dma_start(gp_d.ap()[128 * pt:128 * (pt + 1), :], gsb[:])

            # ---------------- Phase G: gather only (debug) ----------------
            if "G" in stage:
                nc.vector.memset(pred_sb[:], 0.0)
                with tc.tile_pool(name="gat0", bufs=2) as gat0:
                    for b in range(NB):
                        g_sb = gat0.tile([128, GB // 128, JW], F16, tag="g")
                        nc.gpsimd.dma_gather(
                            g_sb[:], gp_d.ap(),
                            idx_sb[:, (GB // 16) * b:(GB // 16) * (b + 1)],
                            GB, GB, JW, transpose=False,
                            single_packet=False).then_inc(gsems[b], 16)
                        nc.vector.tensor_copy(
                            pred_sb[:, 16 * b:16 * b + 1, 0:1],
                            g_sb[:, 0, 0:1])._wait_ge(gsems[b], 16)

            # ---------------- Phase T: TTR only (debug) ----------------
            if "T" in stage:
                with tc.tile_pool(name="gat1", bufs=2) as gat1, \
                     tc.tile_pool(name="scrT", bufs=2) as scrpT:
                    for b in range(NB):
                        g_sb = gat1.tile([128, GB // 128, JW], F16, tag="g")
                        nc.vector.memset(g_sb[:], 0.5)
                        for s in range(GB // 128):
                            k = (GB // 128) * b + s
                            for o in range(3):
                                scr = scrpT.tile([128, HID], F16, tag="scr")
                                nc.vector.scalar_tensor_tensor(
                                    scr[:],
                                    hdd[:, k, :], 0.0,
                                    g_sb[:, s, HID * o:HID * (o + 1)],
                                    AL.bypass, AL.mult,
                                    accum_out=pred_sb[:, k, o:o + 1],
                                )

            # ---------------- Phase E: gather + contraction ----------------
            if "E" in stage:
                with tc.tile_pool(name="gat", bufs=2) as gat, \
                     tc.tile_pool(name="scr", bufs=2) as scrp:
                    for b in range(NB):
                        g_sb = gat.tile([128, GB // 128, JW], F16, tag="g")
                        nc.gpsimd.dma_gather(
                            g_sb[:], gp_d.ap(),
                            idx_sb[:, (GB // 16) * b:(GB // 16) * (b + 1)],
                            GB, GB, JW, transpose=False,
                            single_packet=False).then_inc(gsems[b], 16)
                        for s in range(GB // 128):
                            k = (GB // 128) * b + s
                            for o in range(3):
                                scr = scrp.tile([128, HID], F16, tag="scr")
                                nc.vector.scalar_tensor_tensor(
                                    scr[:],
                                    hdd[:, k, :], 0.0,
                                    g_sb[:, s, HID * o:HID * (o + 1)],
                                    AL.bypass, AL.mult,
                                    accum_out=pred_sb[:, k, o:o + 1],
                                )._wait_ge(gsems[b], 16)
                            if has_b2:
                                nc.vector.tensor_tensor(
                                    pred_sb[:, k, :],
                                    pred_sb[:, k, :],
                                    g_sb[:, s, 768:771],
                                    AL.add)._wait_ge(gsems[b], 16)
            nc.sync.dma_start(
                pred_d.ap().rearrange("(k p) o -> p k o", p=128), pred_sb[:])

    nc.compile()
    return nc


# ---------------- host side ----------------

def pack_w2p(w2: np.ndarray, b2: np.ndarray, has_b2: bool) -> np.ndarray:
    JW = 896 if has_b2 else 768
    w2p = np.zeros((640, JW), np.float16)
    # w2: (256, 1728); k_ref = c*9 + t ; our k' = t*64 + c ; col j = o*256 + h
    w2r = w2.reshape(HID, C, 9, 3)                      # h, c, t, o
    kp = np.transpose(w2r, (2, 1, 3, 0))                # t, c, o, h
    w2p[:576, :768] = kp.reshape(576, 768).astype(np.float16)
    if has_b2:
        b2r = b2.reshape(C, 9, 3)                       # c, t, o
        w2p[:576, 768:771] = np.transpose(b2r, (1, 0, 2)).reshape(576, 3).astype(np.float16)
    return w2p


_NC_CACHE = {}


def _get_nc(has_b2: bool):
    if has_b2 not in _NC_CACHE:
        _NC_CACHE[has_b2] = build_nc(has_b2)
    return _NC_CACHE[has_b2]


def _in_maps(feat, coord, cell, w1, b1, w2, b2, has_b2):
    w2p = pack_w2p(np.asarray(w2, np.float32), np.asarray(b2, np.float32), has_b2)
    w1a = np.zeros((4, HID), np.float32)
    w1a[:3] = np.asarray(w1, np.float32)
    w1a[3] = np.asarray(b1, np.float32)
    in_maps = []
    for core in range(8):
        b, hh = core // 2, core % 2
        sl = slice(hh * QC, (hh + 1) * QC)
        in_maps.append({
            "featb": np.ascontiguousarray(feat[b].reshape(C, HW), np.float32),
            "coords": np.ascontiguousarray(coord[b, sl].reshape(128, 128), np.float32),
            "cells": np.ascontiguousarray(cell[b, sl].reshape(128, 128), np.float32),
            "w1a": w1a,
            "w2p": w2p,
        })
    return in_maps


def kernel(feat, coord, cell, w1, b1, w2, b2):
    feat = np.asarray(feat, np.float32)
    coord = np.asarray(coord, np.float32)
    cell = np.asarray(cell, np.float32)
    B, Q = feat.shape[0], coord.shape[1]
    assert feat.shape == (4, 64, 64, 64) and Q == 16384, (feat.shape, Q)
    has_b2 = bool(np.any(np.asarray(b2)))
    nc = _get_nc(has_b2)
    res = run_bass_kernel_spmd(
        nc, _in_maps(feat, coord, cell, w1, b1, w2, b2, has_b2),
        core_ids=list(range(8)))
    out = np.zeros((B, Q, 3), np.float32)
    for core in range(8):
        b, hh = core // 2, core % 2
        out[b, hh * QC:(hh + 1) * QC] = res.results[core]["pred"]
    return out


def profile(feat, coord, cell, w1, b1, w2, b2):
    """Run once with NTFF tracing; returns exec_time_ns (or None)."""
    feat = np.asarray(feat, np.float32)
    coord = np.asarray(coord, np.float32)
    cell = np.asarray(cell, np.float32)
    has_b2 = bool(np.any(np.asarray(b2)))
    nc = _get_nc(has_b2)
    res = run_bass_kernel_spmd(
        nc, _in_maps(feat, coord, cell, w1, b1, w2, b2, has_b2),
        core_ids=list(range(8)), trace=True)
    return res.exec_time_ns
